# revision 23
# baseline (speedup 1.0000x reference)
"""Trainium2 Bass kernel for nn_ConvolutionLayer (5x5 VALID conv).

Full inputs:  x (16,32,224,224) f32, weight (64,32,5,5) f32, bias (64,) f32
Full output:  (16,64,220,220) f32

Sharding: data-parallel over batch — 2 images per core on 8 cores.

Matmuls run in bfloat16 (tolerance 2e-2 >> bf16 conv error ~2.4e-3;
fp32r measured ~4 cyc/row on HW vs bf16 1 cyc/row). All matmuls use a
uniform (128,128) PE tile config — strip lhsT tiles are zero-padded to
128 rows and group-4 cols to 128 (zeros accumulate harmlessly) — since
mixing (32,x)/(128,64) tile configs was measured to stall the PE ~730ns
per reconfig. Weights live in 15 separate [128,128] SBUF tiles (slices
of one wide tile measurably break LdWeights double-buffering). Per row,
the 3 main matmuls issue before the 3 strips (morder) so the next-block
x DMA has slack. x blocks load 2-per-DMA (dma2, 56 DMAs/pass) and 8
PSUM banks rotate (psbufs=8) — both measured wins.

Per-core algorithm (base layout, from the fp32r original):
  - x stored in SBUF as row-quad blocks [128=(r,c), 452=(img,w)+pad] on
    two grids: G0 blocks start at rows 0 mod 4, G1 blocks at rows 2
    mod 4. For any output row h, the 5-row contraction window (rows
    h..h+4, (kh,c) on partitions) splits into one K=128 "main" matmul
    at base 0 plus one K=32 "strip" matmul — 2 matmuls per kw-group.
  - kw packed into M: groups {kw0,kw2} and {kw1,kw3} (M=128: lo half f
    for the even kw, hi half f for the odd kw) — within a group both
    halves share the same rhs column shift, so all 6 matmuls of a row
    accumulate into ONE psum bank P[128,448]:
       P[f,    w] += z0[w]   (+ z1[w+1] via shift-1 group) (+ z4[w+4], M=64)
       P[64+f, w] += z2[w]   (+ z3[w+1])
    giving out[f,h,w] = P[f, w] + P[64+f, w+2] + bias.
    Shifted reads are single-segment across the padded 452-wide block;
    seam bleed lands only in dead columns 220..223 / 444..447.
  - Epilogue: ACT Identity(+bias) moves the lo half to the staging tile,
    VE adds the hi half (PSUM read at base 64) in place. Staging tile
    [128=(img,f), GH*220] is DMA'd to HBM every GH rows.
"""
import sys

sys.path.insert(0, "/opt/trn_rl_repo")

import numpy as np
import ml_dtypes
import concourse.bacc as bacc
import concourse.mybir as mybir
from concourse.tile import TileContext
from concourse.bass_utils import run_bass_kernel_spmd

F32 = mybir.dt.float32
MMDT = mybir.dt.bfloat16   # matmul operand dtype (PSUM accum stays f32)
NPDT = ml_dtypes.bfloat16

N_CORES = 8
B, C, H, W = 16, 32, 224, 224
F, K = 64, 5
HO, WO = H - K + 1, W - K + 1  # 220, 220
NB = 2                      # images per core
NBLK = H // 4               # 56 row-quad blocks per grid
NW = NB * W                 # 448 data columns per block
NWP = NW + 4                # padded width (shifted reads up to +4)
GH = 10                     # output rows per staging/DMA group

_cache = {}


def _build(reps=1, xbufs=6, psbufs=8, stbufs=3, strips=True, odt=F32,
           uniform=True, wsplit=True, eflat=False, gh=GH, morder=True,
           noepi=False, dma2=True, qk=2):
    nc = bacc.Bacc(trn_type="TRN2")

    xg0 = nc.dram_tensor("xg0", [NBLK, 128, NWP], MMDT, kind="ExternalInput")
    xg1 = nc.dram_tensor("xg1", [NBLK, 128, NWP], MMDT, kind="ExternalInput")
    if uniform:
        wall = nc.dram_tensor("wall", [128, 15 * 128], MMDT,
                              kind="ExternalInput")
    else:
        w02 = nc.dram_tensor("w02", [160, 128], MMDT, kind="ExternalInput")
        w13 = nc.dram_tensor("w13", [160, 128], MMDT, kind="ExternalInput")
        w4 = nc.dram_tensor("w4", [160, 64], MMDT, kind="ExternalInput")
    bias = nc.dram_tensor("bias", [64, 1], F32, kind="ExternalInput")
    out = nc.dram_tensor("out", [NB, F, HO, WO], odt, kind="ExternalOutput")

    with TileContext(nc) as tc:
        with (
            tc.tile_pool(name="wp", bufs=1) as wp,
            tc.tile_pool(name="bp", bufs=1) as bp,
            tc.tile_pool(name="xp", bufs=xbufs) as xp,
            tc.tile_pool(name="pp", bufs=psbufs, space="PSUM") as pp,
            tc.tile_pool(name="op", bufs=stbufs) as op,
        ):
            # ---- weights ----
            # per kw-group g: T1 = Wmat[0:128]@0 (kh0..3), T2 = Wmat[32:160]@0
            # (kh1..4); t345: Wmat[128:160]@0 (kh4), Wmat[0:32]@32, @96 (kh0).
            wt = {}
            if uniform and wsplit:
                wtiles = []
                for i in range(15):
                    t = wp.tile([128, 128], MMDT, tag=f"w{i}")
                    nc.sync.dma_start(out=t[:],
                                      in_=wall[:, i * 128:(i + 1) * 128])
                    wtiles.append(t)

                def wslice(c0):
                    return wtiles[c0 // 128][:]
            elif uniform:
                wtile = wp.tile([128, 15 * 128], MMDT, tag="wall")
                nc.sync.dma_start(out=wtile[:], in_=wall[:, :])

                def wslice(c0):
                    return wtile[:, c0:c0 + 128]
            for name, wd, m in (() if uniform else
                                (("02", w02, 128), ("13", w13, 128),
                                 ("4", w4, 64))):
                t1 = wp.tile([128, m], MMDT, tag=f"t1{name}")
                t2 = wp.tile([128, m], MMDT, tag=f"t2{name}")
                t345 = wp.tile([128, m], MMDT, tag=f"t345{name}")
                nc.sync.dma_start(out=t1[:], in_=wd[0:128, :])
                nc.sync.dma_start(out=t2[:], in_=wd[32:160, :])
                nc.sync.dma_start(out=t345[0:32, :], in_=wd[128:160, :])
                nc.sync.dma_start(out=t345[32:64, :], in_=wd[0:32, :])
                nc.sync.dma_start(out=t345[96:128, :], in_=wd[0:32, :])
                wt[name] = (t1, t2, t345)
            bt = bp.tile([64, 1], F32)
            nc.sync.dma_start(out=bt[:], in_=bias[:])

            g0_tiles, g1_tiles = {}, {}

            def load_block(store, src, b):
                t = xp.tile([128, NWP], MMDT, tag=f"x{'0' if src is xg0 else '1'}")
                nc.sync.dma_start(out=t[:], in_=src[b, :, :])
                store[b] = t

            def load_pair(store, src, j):
                # blocks qk*j .. qk*j+qk-1 in one DMA -> [128, qk, NWP]
                t = xp.tile([128, qk, NWP], MMDT,
                            name="xq", tag=f"q{'0' if src is xg0 else '1'}")
                nc.sync.dma_start(
                    out=t[:, :, :],
                    in_=src[qk * j:qk * j + qk, :, :].rearrange(
                        "b p c -> p b c"))
                store[j] = t

            def get2(store, b):
                return store[b // qk], b % qk

            groups = (("02", 128, 0), ("13", 128, 1), ("4", 64, 4))

            def emit_pass():
                g0_tiles.clear()
                g1_tiles.clear()
                if dma2:
                    for j0 in (0, 1):
                        load_pair(g0_tiles, xg0, j0)
                        load_pair(g1_tiles, xg1, j0)
                else:
                    load_block(g0_tiles, xg0, 0)
                    load_block(g1_tiles, xg1, 0)
                stage = None
                for b in range(55):
                    if dma2:
                        j = b // qk + 2
                        if b % qk == 0 and j <= (NBLK - 1) // qk:
                            load_pair(g0_tiles, xg0, j)
                            load_pair(g1_tiles, xg1, j)
                    else:
                        load_block(g0_tiles, xg0, b + 1)
                        load_block(g1_tiles, xg1, b + 1)
                    for r in range(4):
                        h = 4 * b + r
                        if h % gh == 0:
                            if eflat:
                                stage = op.tile([64, NB, gh, WO], odt,
                                                name="stage", tag="stage")
                            else:
                                stage = op.tile([128, gh * WO], odt,
                                                name="stage", tag="stage")
                        col = (h % gh) * WO

                        if r == 0:
                            mref, wmain = (g0_tiles, b), 0      # kh0..3 -> T1
                            sref, sbase = (g0_tiles, b + 1), 0  # kh4 -> T3
                        elif r == 1:
                            mref, wmain = (g1_tiles, b), 1      # kh1..4 -> T2
                            sref, sbase = (g0_tiles, b), 32     # kh0 -> T4
                        elif r == 2:
                            mref, wmain = (g1_tiles, b), 0
                            sref, sbase = (g1_tiles, b + 1), 0
                        else:
                            mref, wmain = (g0_tiles, b + 1), 1
                            sref, sbase = (g0_tiles, b), 96     # kh0 -> T5
                        if dma2:
                            mt, mi = get2(*mref)
                            st, si = get2(*sref)

                            def mslice(p0, p1, c0, c1, _t=mt, _i=mi):
                                return _t[p0:p1, _i, c0:c1]

                            def sslice(p0, p1, c0, c1, _t=st, _i=si):
                                return _t[p0:p1, _i, c0:c1]
                        else:
                            main = mref[0][mref[1]]
                            strip = sref[0][sref[1]]

                            def mslice(p0, p1, c0, c1, _t=main):
                                return _t[p0:p1, c0:c1]

                            def sslice(p0, p1, c0, c1, _t=strip):
                                return _t[p0:p1, c0:c1]
                        if eflat:
                            assert uniform
                            ps3 = pp.tile([128, NB, W], F32, tag="ps")
                            ps_full = ps3[:, :, :].rearrange(
                                "p n w -> p (n w)")
                        else:
                            ps = pp.tile([128, NW], F32, tag="ps")

                        first = True
                        if uniform:
                            sidx = {0: 2, 32: 3, 96: 4}[sbase]
                            out_ap = ps_full if eflat else ps[0:128, 0:NW]
                            mains_l, strips_l = [], []
                            for gi, (gname, m, sh) in enumerate(groups):
                                base = 5 * 128 * gi
                                mains_l.append(
                                    (wslice(base + wmain * 128),
                                     mslice(0, 128, sh, sh + NW)))
                                if strips:
                                    strips_l.append(
                                        (wslice(base + sidx * 128),
                                         sslice(0, 128, sh, sh + NW)))
                            if morder:
                                mms = mains_l + strips_l
                            else:
                                mms = [t for pair in zip(
                                    mains_l, strips_l + [None] * 3)
                                    for t in pair if t is not None]
                            for i, (lt, rh) in enumerate(mms):
                                nc.tensor.matmul(
                                    out=out_ap, lhsT=lt, rhs=rh,
                                    start=i == 0, stop=i == len(mms) - 1)
                        for gi, (gname, m, sh) in enumerate(
                                () if uniform else groups):
                            t1, t2, t345 = wt[gname]
                            wm = t1 if wmain == 0 else t2
                            rhs_m = mslice(0, 128, sh, sh + NW)
                            rhs_s = sslice(sbase, sbase + 32, sh, sh + NW)
                            out_ap = ps[0:m, 0:NW]
                            last = gi == len(groups) - 1
                            nc.tensor.matmul(out=out_ap, lhsT=wm[0:128, 0:m],
                                             rhs=rhs_m, start=first,
                                             stop=last and not strips)
                            if strips:
                                nc.tensor.matmul(
                                    out=out_ap, lhsT=t345[sbase:sbase + 32, 0:m],
                                    rhs=rhs_s, start=False, stop=last,
                                    tile_position=(sbase, 0))
                            first = False

                        # out[f,h,w] = ps[f, n*224+w] + ps[64+f, n*224+w+2] + b
                        if noepi:
                            # timing ablation: drop ACT/DVE, but keep a
                            # consumer of ps so the bank recycles, and keep
                            # the out DMA (reads stale stage)
                            nc.vector.tensor_copy(
                                out=stage[0:64, col:col + 4],
                                in_=ps[0:64, 0:4])
                            if h % gh == gh - 1:
                                h0 = h - gh + 1
                                nc.sync.dma_start(
                                    out=out[:, :, h0:h0 + gh, :].rearrange(
                                        "n f h w -> (n f) (h w)"),
                                    in_=stage[:],
                                )
                            continue
                        if eflat:
                            hh = h % gh
                            o_lo = stage[0:64, :, hh, 0:WO]
                            nc.scalar.activation(
                                out=o_lo, in_=ps3[0:64, :, 0:WO],
                                func=mybir.ActivationFunctionType.Identity,
                                bias=bt[:], scale=1.0)
                            nc.vector.tensor_add(
                                out=o_lo, in0=ps3[64:128, :, 2:2 + WO],
                                in1=o_lo)
                            if h % gh == gh - 1:
                                h0 = h - gh + 1
                                nc.sync.dma_start(
                                    out=out[:, :, h0:h0 + gh, :].rearrange(
                                        "n f h w -> f n (h w)"),
                                    in_=stage[0:64, :, :, :],
                                )
                            continue
                        for n in range(NB):
                            o_lo = stage[64 * n:64 * n + 64, col:col + WO]
                            nc.scalar.activation(
                                out=o_lo, in_=ps[0:64, n * W:n * W + WO],
                                func=mybir.ActivationFunctionType.Identity,
                                bias=bt[:], scale=1.0)
                            nc.vector.tensor_add(
                                out=o_lo,
                                in0=ps[64:128, n * W + 2:n * W + 2 + WO],
                                in1=o_lo)

                        if h % gh == gh - 1:
                            h0 = h - gh + 1
                            nc.sync.dma_start(
                                out=out[:, :, h0:h0 + gh, :].rearrange(
                                    "n f h w -> (n f) (h w)"),
                                in_=stage[:],
                            )

            for _ in range(reps):
                emit_pass()

    nc.finalize()
    return nc


def _prep_core(xs, weight, bias):
    """xs: (2,32,224,224) -> per-core input map."""
    def _grid(arr):
        g = arr.astype(NPDT).reshape(NB, C, NBLK, 4, W).transpose(2, 3, 1, 0, 4)
        o = np.zeros((NBLK, 128, NWP), NPDT)
        o[:, :, :NW] = g.reshape(NBLK, 128, NW)
        return o

    g0 = _grid(xs)
    xpad = np.concatenate(
        [xs[:, :, 2:, :], np.zeros((NB, C, 2, W), np.float32)], axis=2)
    g1 = _grid(xpad)
    # Wmat[32*kh + c, j*64 + f] = weight[f, c, kh, kw_j]
    wm = weight.astype(NPDT).transpose(2, 1, 0, 3).reshape(160, 64, 5)
    w02 = np.concatenate([wm[:, :, 0], wm[:, :, 2]], axis=1)
    w13 = np.concatenate([wm[:, :, 1], wm[:, :, 3]], axis=1)
    w4 = np.ascontiguousarray(wm[:, :, 4])

    # uniform-tile weight wall: per group g, 5 [128,128] lhsT tiles
    # (T1 kh0..3, T2 kh1..4, W3 kh4@rows0:32, W4 kh0@32:64, W5 kh0@96:128),
    # zero-padded so every matmul is a full (128,128) PE tile.
    wall = np.zeros((128, 15 * 128), NPDT)
    z64 = np.zeros((160, 64), NPDT)
    for gi, Wg in enumerate((w02, w13,
                             np.concatenate([wm[:, :, 4], z64], axis=1))):
        base = 5 * 128 * gi
        wall[:, base:base + 128] = Wg[0:128]
        wall[:, base + 128:base + 256] = Wg[32:160]
        wall[0:32, base + 256:base + 384] = Wg[128:160]
        wall[32:64, base + 384:base + 512] = Wg[0:32]
        wall[96:128, base + 512:base + 640] = Wg[0:32]
    return {
        "xg0": g0,
        "xg1": g1,
        "w02": np.ascontiguousarray(w02),
        "w13": np.ascontiguousarray(w13),
        "w4": w4,
        "wall": wall,
        "bias": bias.reshape(64, 1).astype(np.float32),
    }


def kernel(x, weight, bias, _profile=False):
    x = np.asarray(x, dtype=np.float32)
    weight = np.asarray(weight, dtype=np.float32)
    bias = np.asarray(bias, dtype=np.float32)

    if "nc" not in _cache:
        _cache["nc"] = _build()
    nc = _cache["nc"]

    in_maps = [
        _prep_core(x[NB * i:NB * i + NB], weight, bias) for i in range(N_CORES)
    ]
    res = run_bass_kernel_spmd(
        nc, in_maps, core_ids=list(range(N_CORES)), trace=_profile)
    out = np.concatenate([r["out"] for r in res.results], axis=0)
    if _profile:
        _cache["last_results"] = res
    return out


if __name__ == "__main__":
    rng = np.random.default_rng(0)
    x = rng.standard_normal((B, C, H, W), dtype=np.float32)
    w = rng.standard_normal((F, C, K, K), dtype=np.float32)
    bv = rng.standard_normal((F,), dtype=np.float32)
    o = kernel(x, w, bv)
    print("output shape:", o.shape, o.dtype)

